# revision 36
# baseline (speedup 1.0000x reference)
"""Trainium2 Bass kernel for a transformer decoder layer (self-attn + cross-attn + FFN).

Sharding: 8 cores, data-parallel over (batch, seq): core c handles batch c//4,
rows (c%4)*512:(c%4+1)*512. No collectives; the K projections (which need the
full 2048-token context) are computed replicated per core.

v2 design notes:
  - all matmul operands bf16 (weights cast host-side); PSUM/LN/residual fp32.
  - weights DMA'd as whole [1024,1024]-sized blocks through a 3-deep rotating
    pool tag, prefetched one phase ahead (ffW1/ffW2 stream as 4 blocks each).
  - attention scores transposed s^T[m(part), l(free)]; encoder mask folded as
    per-partition bias into the Exp activation; softmax denominator comes from
    a ones-column appended to K-natural (66-wide) in the value matmul, then
    Z is broadcast via a 1x64 ones matmul and applied with a DVE divide.
  - output projection contracts the full 128-partition head-pair tile.
  - FFN gelu runs as a single scalar-engine Gelu activation per h-tile.
"""

import os
import sys

sys.path.insert(0, "/opt/trn_rl_repo")

import numpy as np
import ml_dtypes

import concourse.bass as bass
import concourse.bacc as bacc
import concourse.mybir as mybir
import concourse.tile as tile
from concourse.bass_utils import run_bass_kernel_spmd
from concourse.masks import make_identity

dt = mybir.dt
AF = mybir.ActivationFunctionType
ALU = mybir.AluOpType

P = 128
D = 1024          # d_model
H = 16            # heads
HD = 64           # head dim
MLP = 4096
B, L, M = 2, 2048, 2048
NCORES = 8
GROUPS = 4        # cores per batch
R = L // GROUPS   # 512 rows per core
LT = R // P       # 4 l-tiles per core
DTL = D // P      # 8 d-tiles
CT = D // P       # 8 c-tiles
MT = M // P       # 16 m-tiles
HT = MLP // P     # 32 hidden tiles
NK = 512          # matmul free-dim chunk
MC = M // NK      # 4 context chunks
KNW = 65          # kn block width: 64 hd + 1 ones
EPS = 1e-5

_PROGRAM_CACHE = {}
_PHASE = int(os.environ.get("KPHASE", "5"))  # 1=QT 2=ctxT 3=x1 4=x2 5=full


def _build_program(trivial_affine, trivial_ffb):
    nc = bacc.Bacc(None)
    f32 = dt.float32
    bf = dt.bfloat16

    def din(name, shape, d=bf):
        return nc.declare_dram_parameter(name, list(shape), d, isOutput=False)

    xTq_d = din("xTq", [D, R])              # this core's columns of x^T
    xT_d = din("xT", [D, M])                # full batch x^T (for K1)
    encT_d = din("encT", [D, M])            # full batch enc^T (for K2)
    xrows_d = din("xrows", [R, D], f32)     # natural rows (residual)
    m01t_d = din("m01t", [P, MT], f32)      # 0 where masked, else 1 (tiled)
    m01b_d = din("m01b", [P, M])            # same, broadcast across partitions
    q1W_d = din("q1W", [D, D]); w1W_d = din("w1W", [D, D]); o1W_d = din("o1W", [D, D])
    q2W_d = din("q2W", [D, D]); w2W_d = din("w2W", [D, D]); o2W_d = din("o2W", [D, D])
    ffW1_d = din("ffW1", [D, MLP]); ffW2_d = din("ffW2", [MLP, D])
    ffb1h_d = din("ffb1h", [P, HT], f32)    # ffb1 tiled [P, ht]
    gb_d = {}
    if not trivial_affine:
        for nm in ("g1", "b1", "g2", "b2", "g3", "b3"):
            gb_d[nm] = din(nm + "b", [P, D], f32)
    if not trivial_ffb:
        ffb2b_d = din("ffb2b", [P, D], f32)
    out_d = nc.declare_dram_parameter("out", [R, D], f32, isOutput=True)

    lp = nc.allow_low_precision(reason="bf16 matmul staging")
    lp.__enter__()
    with tile.TileContext(nc) as tc:
        cpool = tc.alloc_tile_pool(name="const", bufs=1)
        small = tc.alloc_tile_pool(name="small", bufs=3)
        sbP = tc.alloc_tile_pool(name="sbP", bufs=1)

        ident_f = cpool.tile([P, P], f32)
        make_identity(nc, ident_f[:])
        ident_b = cpool.tile([P, P], bf)
        nc.vector.tensor_copy(ident_b[:], ident_f[:])
        ones_b = cpool.tile([1, HD], bf)
        nc.vector.memset(ones_b[:], 1.0)
        m01t_t = cpool.tile([P, MT], f32)
        nc.sync.dma_start(m01t_t[:], m01t_d[:])
        m01b_t = cpool.tile([P, M], bf)
        nc.sync.dma_start(m01b_t[:], m01b_d[:])
        if not trivial_ffb:
            ffb1h_t = cpool.tile([P, HT], f32)
            nc.sync.dma_start(ffb1h_t[:], ffb1h_d[:])

        # weight blocks: [P, DTL, D] bf16 (16KB/partition), 3-deep rotation
        def wblock(name, dram, sub=None):
            t = sbP.tile([P, DTL, D], bf, tag="W", bufs=3, name=name)
            if sub is None:
                nc.sync.dma_start(t[:], dram.rearrange("(dt p) c -> p dt c", p=P))
            else:
                nc.sync.dma_start(t[:], sub)
            return t

        # eviction engine alternation (PSUM fp32 -> SBUF bf16/f32)
        ev_par = [0]

        def evict(dst, src):
            if ev_par[0] % 2 == 0:
                nc.vector.tensor_copy(dst, src)
            else:
                nc.scalar.copy(dst, src)
            ev_par[0] += 1

        def layernorm(rsb, out_nat, gkey):
            """out_nat [P, D] = LN(rsb) * g + b.  Trashes rsb."""
            st = small.tile([P, 2, 6], f32, tag="ln_st")
            nc.vector.bn_stats(st[:, 0, :], rsb[:, 0:512])
            nc.vector.bn_stats(st[:, 1, :], rsb[:, 512:1024])
            mv = small.tile([P, 2], f32, tag="ln_mv")
            nc.vector.bn_aggr(mv[:], st[:])
            t = small.tile([P, 1], f32, tag="ln_t")
            nc.vector.tensor_scalar_add(t[:], mv[:, 1:2], EPS)
            s = small.tile([P, 1], f32, tag="ln_s")
            nc.scalar.sqrt(s[:], t[:])
            r0 = small.tile([P, 1], f32, tag="ln_r0")
            nc.vector.reciprocal(r0[:], s[:])
            # one Newton step: r1 = r0 * (1.5 - 0.5 * t * r0^2)
            u = small.tile([P, 1], f32, tag="ln_u")
            nc.vector.tensor_tensor(out=u[:], in0=t[:], in1=r0[:], op=ALU.mult)
            nc.vector.tensor_tensor(out=u[:], in0=u[:], in1=r0[:], op=ALU.mult)
            nc.vector.tensor_scalar(u[:], u[:], -0.5, 1.5, ALU.mult, ALU.add)
            r1 = small.tile([P, 1], f32, tag="ln_r1")
            nc.vector.tensor_tensor(out=r1[:], in0=r0[:], in1=u[:], op=ALU.mult)
            nc.vector.tensor_scalar(rsb[:], rsb[:], mv[:, 0:1], None, ALU.subtract)
            if trivial_affine:
                nc.vector.tensor_scalar(out_nat[:], rsb[:], r1[:], None, ALU.mult)
            else:
                g_t = small.tile([P, D], f32, tag="ln_g", bufs=2)
                nc.sync.dma_start(g_t[:], gb_d["g" + gkey][:])
                b_t = small.tile([P, D], f32, tag="ln_b", bufs=2)
                nc.sync.dma_start(b_t[:], gb_d["b" + gkey][:])
                nc.vector.tensor_scalar(rsb[:], rsb[:], r1[:], None, ALU.mult)
                nc.vector.tensor_tensor(out=rsb[:], in0=rsb[:], in1=g_t[:], op=ALU.mult)
                nc.vector.tensor_tensor(out=out_nat[:], in0=rsb[:], in1=b_t[:], op=ALU.add)

        xrows_t = sbP.tile([P, LT, D], f32, tag="xnat", bufs=2, name="xrows")
        nc.sync.dma_start(xrows_t[:], xrows_d.rearrange("(lt p) d -> p lt d", p=P))

        def qproj(wq, xqa, QT, pp):
            """QT[P, CT, R] bf16 = (x @ qW)^T for this core's rows."""
            for co in range(2):
                for ct in range(4):
                    ps = pp.tile([P, NK], f32, tag="pq", bufs=8, name="pq")
                    for dti in range(DTL):
                        nc.tensor.matmul(
                            ps[:], wq[:, dti, co * NK + ct * P:co * NK + ct * P + P],
                            xqa[:, dti, :],
                            start=(dti == 0), stop=(dti == DTL - 1))
                    evict(QT[:, co * 4 + ct, :], ps[:])

        def kproj_units(wk, kT_dram, sink, pp, pq_bufs):
            """K projection generator: yields after each 2-matmul unit.

            sink(g, mc, ps) consumes each finished [P, NK] PSUM group.
            With pq_bufs=8 all 8 groups of an mc are open at once (proj
            phase); with fewer bufs the group loop still works, just with
            tighter rotation.
            """
            src = kT_dram.rearrange("(dt p) m -> p dt m", p=P)
            for mc in range(MC):
                if pq_bufs >= 8:
                    pss = [pp.tile([P, NK], f32, tag="pq", bufs=pq_bufs,
                                   name=f"pk{g}") for g in range(8)]
                    for half in range(4):
                        xc = sbP.tile([P, 2, NK], bf, tag="xc", bufs=2, name="xc")
                        nc.sync.dma_start(
                            xc[:], src[:, 2 * half:2 * half + 2, bass.ts(mc, NK)])
                        for g in range(8):
                            for i2 in range(2):
                                co, ct = g // 4, g % 4
                                nc.tensor.matmul(
                                    pss[g][:],
                                    wk[:, 2 * half + i2,
                                       co * NK + ct * P:co * NK + ct * P + P],
                                    xc[:, i2, :],
                                    start=(half == 0 and i2 == 0),
                                    stop=(half == 3 and i2 == 1))
                            yield
                    for g in range(8):
                        sink(g, mc, pss[g])
                        yield
                else:
                    # group pairs with chunk reload: only 2 PSUM banks and
                    # one small moving tile live at a time (filler mode; the
                    # extra DMA re-reads ride the idle DMA engine)
                    for gp in range(4):
                        pss = [pp.tile([P, NK], f32, tag="pq", bufs=pq_bufs,
                                       name=f"pk{g}") for g in range(2)]
                        for half in range(4):
                            xc = sbP.tile([P, 2, NK], bf, tag="xc", bufs=2,
                                          name="xc")
                            nc.sync.dma_start(
                                xc[:],
                                src[:, 2 * half:2 * half + 2, bass.ts(mc, NK)])
                            for gi in range(2):
                                g = 2 * gp + gi
                                co, ct = g // 4, g % 4
                                for i2 in range(2):
                                    nc.tensor.matmul(
                                        pss[gi][:],
                                        wk[:, 2 * half + i2,
                                           co * NK + ct * P:co * NK + ct * P + P],
                                        xc[:, i2, :],
                                        start=(half == 0 and i2 == 0),
                                        stop=(half == 3 and i2 == 1))
                                yield
                        for gi in range(2):
                            sink(2 * gp + gi, mc, pss[gi])
                            yield

        def kproj(wk, kT_dram, KT, pp, masked=False):
            def sink(g, mc, ps):
                if masked:
                    # fold the encoder mask in: zero masked key columns
                    nc.vector.tensor_tensor(out=KT[:, g, bass.ts(mc, NK)],
                                            in0=ps[:],
                                            in1=m01b_t[:, bass.ts(mc, NK)],
                                            op=ALU.mult)
                else:
                    evict(KT[:, g, bass.ts(mc, NK)], ps[:])
            for _ in kproj_units(wk, kT_dram, sink, pp, 8):
                pass

        def dbg_out(src_ap):
            stg = sbP.tile([P, LT, D], f32, tag="dbg", name="dbg")
            nc.vector.tensor_copy(stg[:].rearrange("p a b -> p (a b)"), src_ap)
            nc.sync.dma_start(out_d.rearrange("(lt p) d -> p lt d", p=P), stg[:])

        def attention(qW_dram, wW_dram, oW_dram, q_src, kT_dram, use_mask,
                      resid_nat, x_out, gkey, want_xt, wq=None, wk=None,
                      prefetch=None, filler_factory=None, kt_src=None):
            """One MHA block + residual + LN.

            q_src: DRAM handle [D, R] or sbuf tile [P, DTL, R] bf16
            resid_nat/x_out: sbuf [P, LT, D] f32
            prefetch: callback emitted mid-core (weight DMA issue points)
            filler_factory(pa): generator of independent PE work interleaved
                into the attention core (keeps the PE P-state high)
            kt_src: DRAM scratch holding precomputed KT (skips kproj)
            returns xT_out sbuf [P, DTL, R] bf16 if want_xt
            """
            if wq is None:
                wq = wblock("wq", qW_dram)
            if wk is None and kt_src is None:
                wk = wblock("wk", wW_dram)

            if isinstance(q_src, bass.DRamTensorHandle):
                xqa = sbP.tile([P, DTL, NK], bf, tag="xT_fam", bufs=2, name="xqa")
                nc.sync.dma_start(xqa[:], q_src.rearrange("(dt p) r -> p dt r", p=P))
            else:
                xqa = q_src

            QT = sbP.tile([P, CT, R], bf, tag="QT", bufs=1, name="QT")
            KT = sbP.tile([P, CT, M], bf, tag="KT", bufs=1, name="KT")
            if kt_src is not None:
                nc.sync.dma_start(KT[:], kt_src[:])
            with tc.tile_pool(name="ps_proj", bufs=1, space="PSUM") as pp:
                qproj(wq, xqa, QT, pp)
                if kt_src is None:
                    kproj(wk, kT_dram, KT, pp, masked=use_mask)

            if _PHASE == 1:
                dbg_out(QT[:].rearrange("p a b -> p (a b)"))
                return None

            wo = wblock("wo", oW_dram)
            if prefetch is not None:
                prefetch()

            ctxT = sbP.tile([P, H // 2, R], bf, tag="ctxT", bufs=1, name="ctxT")

            def build_kn(hp, pa):
                kn = sbP.tile([P, MT, 2, KNW], bf, tag="kn", bufs=2, name="kn")
                if use_mask:
                    # ones column carries the mask so Z skips masked keys
                    for j in range(2):
                        nc.vector.tensor_copy(
                            kn[:, :, j, HD:KNW].rearrange("p a b -> p (a b)"),
                            m01t_t[:])
                else:
                    nc.vector.memset(kn[:, :, :, HD:KNW], 1.0)
                for mh in range(4):
                    tp = pa.tile([P, 4, P], bf, tag="knt", bufs=1, name="tp")
                    for j in range(4):
                        nc.tensor.transpose(
                            tp[:, j, :], KT[:, hp, bass.ts(4 * mh + j, P)],
                            ident_b[:])
                    nc.vector.tensor_copy(
                        kn[:, 4 * mh:4 * mh + 4, :, 0:HD],
                        tp[:].rearrange("p m (j h) -> p m j h", h=HD))
                return kn

            # --- attention core: head pairs, scores transposed ---
            s2_bufs = 1  # [P,2,2,NK] spans 4 banks; 8-bank budget allows one
            with tc.tile_pool(name="ps_attn", bufs=1, space="PSUM") as pa:
                filler = filler_factory(pa) if filler_factory is not None else None
                fill_done = filler is None

                def fill(n):
                    nonlocal fill_done
                    if fill_done:
                        return
                    try:
                        for _ in range(n):
                            next(filler)
                    except StopIteration:
                        fill_done = True

                kn_cur = build_kn(0, pa)
                for hp in range(H // 2):
                    kn_next = build_kn(hp + 1, pa) if hp + 1 < H // 2 else None
                    ctxp = [pa.tile([P, NK], f32, tag="ctx", bufs=2, name=f"ctx{j}")
                            for j in range(2)]
                    for mp in range(MT // 2):
                        s4 = pa.tile([P, 2, 2, NK], f32, tag="s2", bufs=s2_bufs)
                        for mi in range(2):
                            for j in range(2):
                                nc.tensor.matmul(
                                    s4[:, mi, j, :],
                                    KT[bass.ts(j, HD), hp, bass.ts(2 * mp + mi, P)],
                                    QT[bass.ts(j, HD), hp, :],
                                    start=True, stop=True)
                        p4 = sbP.tile([P, 2, 2, NK], bf, tag="p2", bufs=1,
                                      name="p4")
                        nc.scalar.activation(p4[:], s4[:], AF.Exp, scale=0.125)
                        fill(2)
                        for mi in range(2):
                            mt = 2 * mp + mi
                            for j in range(2):
                                nc.tensor.matmul(
                                    ctxp[j][0:KNW, :],
                                    kn_cur[:, mt, j, :], p4[:, mi, j, :],
                                    start=(mt == 0), stop=(mt == MT - 1))
                    # Evict ctx unnormalized (frees the banks fast), then
                    # broadcast Z via PE into a spare bank, reciprocal
                    # PSUM->bf16, and scale ctxT in place.
                    pz = pa.tile([P, NK], f32, tag="pz", bufs=1, name="pz")
                    for j in range(2):
                        dnb = small.tile([1, NK], bf, tag="dnb", bufs=2)
                        nc.vector.tensor_copy(dnb[:], ctxp[j][HD:HD + 1, :])
                        nc.vector.tensor_copy(ctxT[bass.ts(j, HD), hp, :],
                                              ctxp[j][0:HD, :])
                        nc.tensor.matmul(pz[bass.ts(j, HD), :], ones_b[:], dnb[:],
                                         start=True, stop=True)
                    zbr = small.tile([P, NK], bf, tag="zbr", bufs=2)
                    nc.vector.reciprocal(zbr[:], pz[:])
                    for j in range(2):
                        nc.vector.tensor_tensor(out=ctxT[bass.ts(j, HD), hp, :],
                                                in0=ctxT[bass.ts(j, HD), hp, :],
                                                in1=zbr[bass.ts(j, HD), :],
                                                op=ALU.mult)
                    kn_cur = kn_next
                fill(1000)
            if _PHASE == 2:
                dbg_out(ctxT[:].rearrange("p a b -> p (a b)"))
                return None

            # --- output projection (full 128-contract) + residual + LN ---
            xT_out = None
            if want_xt:
                xT_out = sbP.tile([P, DTL, R], bf, tag="xT_fam", bufs=2,
                                  name="xT_out")
            with tc.tile_pool(name="ps_o", bufs=1, space="PSUM") as po:
                for lt in range(LT):
                    rsb = small.tile([P, D], f32, tag="rsb", bufs=1)
                    for dc in range(2):
                        ps = po.tile([P, NK], f32, tag="po", bufs=4, name="ps_o")
                        for i in range(H // 2):
                            nc.tensor.matmul(
                                ps[:],
                                ctxT[:, i, bass.ts(lt, P)],
                                wo[:, i, bass.ts(dc, NK)],
                                start=(i == 0), stop=(i == H // 2 - 1))
                        nc.vector.tensor_tensor(out=rsb[:, bass.ts(dc, NK)],
                                                in0=ps[:],
                                                in1=resid_nat[:, lt, bass.ts(dc, NK)],
                                                op=ALU.add)
                    layernorm(rsb, x_out[:, lt, :], gkey)
                # transposes emitted after all LNs so the PE never waits on
                # a just-computed LN result
                if want_xt:
                    for lt in range(LT):
                        for half in range(2):
                            tp = po.tile([P, 4, P], f32, tag="pt", bufs=2)
                            for i in range(4):
                                nc.tensor.transpose(
                                    tp[:, i, :],
                                    x_out[:, lt, bass.ts(4 * half + i, P)],
                                    ident_f[:])
                            nc.vector.tensor_copy(
                                xT_out[:, 4 * half:4 * half + 4, bass.ts(lt, P)],
                                tp[:])
            return xT_out

        # ---------------- program ----------------
        dpool = tc.alloc_tile_pool(name="dscr", bufs=1, space="DRAM")
        kt2scr = dpool.tile([P, CT, M], bf, tag="kt2", bufs=1, name="kt2scr")

        wq1 = wblock("wq1", q1W_d)
        wk1 = wblock("wk1", w1W_d)

        pf_state = {}

        def prefetch1():
            pf_state["wq2"] = wblock("wq2", q2W_d)
            pf_state["wk2"] = wblock("wk2", w2W_d)

        def k2_sink(g, mc, ps):
            stg = sbP.tile([P, NK], bf, tag="kstg", bufs=2, name="kstg")
            nc.vector.tensor_copy(stg[:], ps[:])
            nc.sync.dma_start(kt2scr[:, g, bass.ts(mc, NK)], stg[:])

        def k2_filler(pa):
            return kproj_units(pf_state["wk2"], encT_d, k2_sink, pa, 2)

        x1 = sbP.tile([P, LT, D], f32, tag="xnat", bufs=2, name="x1")
        x1T = attention(q1W_d, w1W_d, o1W_d, xTq_d, xT_d, False,
                        xrows_t, x1, "1", want_xt=(_PHASE > 3),
                        wq=wq1, wk=wk1, prefetch=prefetch1)
        if _PHASE == 3:
            nc.sync.dma_start(out_d.rearrange("(lt p) d -> p lt d", p=P), x1[:])

        if _PHASE >= 4:
            f1b = {}

            def prefetch2():
                f1b[0] = wblock("f10", ffW1_d,
                                ffW1_d.rearrange("(dt p) h -> p dt h", p=P)[:, :, 0:D])
                f1b[1] = wblock("f11", ffW1_d,
                                ffW1_d.rearrange("(dt p) h -> p dt h", p=P)[:, :, D:2 * D])

            x2 = sbP.tile([P, LT, D], f32, tag="xnat", bufs=2, name="x2")
            x2T = attention(q2W_d, w2W_d, o2W_d, x1T, encT_d, True,
                            x1, x2, "2", want_xt=(_PHASE > 4),
                            wq=pf_state["wq2"], wk=pf_state["wk2"],
                            prefetch=prefetch2)
            if _PHASE == 4:
                nc.sync.dma_start(out_d.rearrange("(lt p) d -> p lt d", p=P), x2[:])

        # --- FFN ---
        if _PHASE >= 5:
            ff1r = ffW1_d.rearrange("(dt p) h -> p dt h", p=P)
            ff2r = ffW2_d.rearrange("(ht p) d -> p ht d", p=P)
            x3 = sbP.tile([P, LT, D], f32, tag="xnat", bufs=2, name="x3")
            hT = sbP.tile([P, HT, R], bf, tag="hT", bufs=1, name="hT")
            f2b = {}
            with tc.tile_pool(name="ps_ff1", bufs=1, space="PSUM") as pf1:
                for q in range(4):
                    if q >= 2:
                        f1b[q] = wblock(f"f1{q}", ffW1_d,
                                        ff1r[:, :, q * D:(q + 1) * D])
                    if q == 3:
                        f2b[0] = wblock("f20", ffW2_d, ff2r[:, 0:DTL, :])
                    wt = f1b[q]
                    for hc in range(DTL):
                        ht = q * DTL + hc
                        ps = pf1.tile([P, R], f32, tag="ph", bufs=2)
                        for dti in range(DTL):
                            nc.tensor.matmul(ps[:], wt[:, dti, bass.ts(hc, P)],
                                             x2T[:, dti, :],
                                             start=(dti == 0), stop=(dti == DTL - 1))
                        bias = 0.0 if trivial_ffb else ffb1h_t[:, ht:ht + 1]
                        nc.scalar.activation(hT[:, ht, :], ps[:], AF.Gelu,
                                             bias=bias, scale=1.0)

            with tc.tile_pool(name="ps_ff2", bufs=1, space="PSUM") as pf2:
                psl = [pf2.tile([P, 2, NK], f32, tag=f"pf{lt}", bufs=1,
                                name=f"pf{lt}") for lt in range(LT)]
                for q in range(4):
                    if q < 3:
                        # stream next ffW2 block; buffer freed by earlier reads
                        f2b[q + 1] = wblock(f"f2{q + 1}", ffW2_d,
                                            ff2r[:, (q + 1) * DTL:(q + 2) * DTL, :])
                    wt = f2b[q]
                    for hc in range(DTL):
                        ht = q * DTL + hc
                        for lt in range(LT):
                            for dc in range(2):
                                nc.tensor.matmul(
                                    psl[lt][:, dc, :],
                                    hT[:, ht, bass.ts(lt, P)],
                                    wt[:, hc, bass.ts(dc, NK)],
                                    start=(ht == 0), stop=(ht == HT - 1))
                for lt in range(LT):
                    rsb3 = small.tile([P, D], f32, tag="rsb", bufs=1)
                    for dc in range(2):
                        if trivial_ffb:
                            nc.vector.tensor_tensor(out=rsb3[:, bass.ts(dc, NK)],
                                                    in0=psl[lt][:, dc, :],
                                                    in1=x2[:, lt, bass.ts(dc, NK)],
                                                    op=ALU.add)
                        else:
                            ffb2c = small.tile([P, NK], f32, tag="ffb2", bufs=2)
                            nc.sync.dma_start(ffb2c[:], ffb2b_d[:, bass.ts(dc, NK)])
                            nc.vector.tensor_tensor(out=rsb3[:, bass.ts(dc, NK)],
                                                    in0=psl[lt][:, dc, :],
                                                    in1=ffb2c[:],
                                                    op=ALU.add)
                            nc.vector.tensor_tensor(out=rsb3[:, bass.ts(dc, NK)],
                                                    in0=rsb3[:, bass.ts(dc, NK)],
                                                    in1=x2[:, lt, bass.ts(dc, NK)],
                                                    op=ALU.add)
                    layernorm(rsb3, x3[:, lt, :], "3")
            nc.sync.dma_start(out_d.rearrange("(lt p) d -> p lt d", p=P), x3[:])

        sbP.release()
        small.release()
        cpool.release()

    lp.__exit__(None, None, None)
    nc.compile()
    return nc


def _host_prep(inputs):
    bfd = ml_dtypes.bfloat16
    x = np.ascontiguousarray(np.asarray(inputs["x"], np.float32))
    enc = np.ascontiguousarray(np.asarray(inputs["enc_output"], np.float32))
    mask = np.asarray(inputs["mask"])

    n = np.arange(D) // HD
    d = np.arange(D) % HD
    perm = d * H + n

    def pw(q, w, o):
        return (np.ascontiguousarray(np.asarray(q, np.float32)[:, perm].astype(bfd)),
                np.ascontiguousarray(np.asarray(w, np.float32)[:, perm].astype(bfd)),
                np.ascontiguousarray(np.asarray(o, np.float32)[perm, :].astype(bfd)))

    q1W, w1W, o1W = pw(inputs["q1W"], inputs["w1W"], inputs["o1W"])
    q2W, w2W, o2W = pw(inputs["q2W"], inputs["w2W"], inputs["o2W"])
    ffW1 = np.ascontiguousarray(np.asarray(inputs["ffW1"], np.float32).astype(bfd))
    ffW2 = np.ascontiguousarray(np.asarray(inputs["ffW2"], np.float32).astype(bfd))
    ffb1 = np.asarray(inputs["ffb1"], np.float32)
    ffb2 = np.asarray(inputs["ffb2"], np.float32)
    g = {k: np.asarray(inputs[k], np.float32)
         for k in ("g1", "b1", "g2", "b2", "g3", "b3")}

    trivial_affine = all(
        np.all(g[f"g{i}"] == 1.0) and np.all(g[f"b{i}"] == 0.0) for i in (1, 2, 3))
    trivial_ffb = bool(np.all(ffb1 == 0.0) and np.all(ffb2 == 0.0))

    mask01 = np.where(mask[:, 0, :, 0], np.float32(0.0),
                      np.float32(1.0)).astype(np.float32)

    xT = [np.ascontiguousarray(x[b].T.astype(bfd)) for b in range(B)]
    encT = [np.ascontiguousarray(enc[b].T.astype(bfd)) for b in range(B)]
    m01t = [np.ascontiguousarray(mask01[b].reshape(MT, P).T) for b in range(B)]
    m01b = [np.ascontiguousarray(
        np.broadcast_to(mask01[b].astype(bfd), (P, M))) for b in range(B)]

    in_maps = []
    for c in range(NCORES):
        b, grp = c // GROUPS, c % GROUPS
        im = {
            "xTq": np.ascontiguousarray(xT[b][:, grp * R:(grp + 1) * R]),
            "xT": xT[b],
            "encT": encT[b],
            "xrows": np.ascontiguousarray(x[b, grp * R:(grp + 1) * R, :]),
            "m01t": m01t[b], "m01b": m01b[b],
            "q1W": q1W, "w1W": w1W, "o1W": o1W,
            "q2W": q2W, "w2W": w2W, "o2W": o2W,
            "ffW1": ffW1, "ffW2": ffW2,
            "ffb1h": np.ascontiguousarray(ffb1.reshape(HT, P).T),
        }
        if not trivial_affine:
            for k in ("g1", "b1", "g2", "b2", "g3", "b3"):
                im[k + "b"] = np.ascontiguousarray(
                    np.broadcast_to(g[k], (P, D)).astype(np.float32))
        if not trivial_ffb:
            im["ffb2b"] = np.ascontiguousarray(
                np.broadcast_to(ffb2, (P, D)).astype(np.float32))
        in_maps.append(im)
    return in_maps, trivial_affine, trivial_ffb


def kernel(**inputs) -> np.ndarray:
    in_maps, trivial_affine, trivial_ffb = _host_prep(inputs)
    key = (trivial_affine, trivial_ffb)
    if key not in _PROGRAM_CACHE:
        _PROGRAM_CACHE[key] = _build_program(*key)
    nc = _PROGRAM_CACHE[key]
    res = run_bass_kernel_spmd(nc, in_maps, list(range(NCORES)))
    out = np.empty((B, L, D), np.float32)
    for c in range(NCORES):
        b, grp = c // GROUPS, c % GROUPS
        out[b, grp * R:(grp + 1) * R, :] = res.results[c]["out"]
    return out


# revision 38
# speedup vs baseline: 1.3432x; 1.3432x over previous
"""Trainium2 Bass kernel for a transformer decoder layer (self-attn + cross-attn + FFN).

Sharding: 8 cores, data-parallel over (batch, seq): core c handles batch c//4,
rows (c%4)*512:(c%4+1)*512. No collectives; the K projections (which need the
full 2048-token context) are computed replicated per core.

v2 design notes:
  - all matmul operands bf16 (weights cast host-side); PSUM/LN/residual fp32.
  - weights DMA'd as whole [1024,1024]-sized blocks through a 3-deep rotating
    pool tag, prefetched one phase ahead (ffW1/ffW2 stream as 4 blocks each).
  - attention scores transposed s^T[m(part), l(free)]; encoder mask folded as
    per-partition bias into the Exp activation; softmax denominator comes from
    a ones-column appended to K-natural (66-wide) in the value matmul, then
    Z is broadcast via a 1x64 ones matmul and applied with a DVE divide.
  - output projection contracts the full 128-partition head-pair tile.
  - FFN gelu runs as a single scalar-engine Gelu activation per h-tile.
"""

import os
import sys

sys.path.insert(0, "/opt/trn_rl_repo")

import numpy as np
import ml_dtypes

import concourse.bass as bass
import concourse.bacc as bacc
import concourse.mybir as mybir
import concourse.tile as tile
from concourse.bass_utils import run_bass_kernel_spmd
from concourse.masks import make_identity

dt = mybir.dt
AF = mybir.ActivationFunctionType
ALU = mybir.AluOpType

P = 128
D = 1024          # d_model
H = 16            # heads
HD = 64           # head dim
MLP = 4096
B, L, M = 2, 2048, 2048
NCORES = 8
GROUPS = 4        # cores per batch
R = L // GROUPS   # 512 rows per core
LT = R // P       # 4 l-tiles per core
DTL = D // P      # 8 d-tiles
CT = D // P       # 8 c-tiles
MT = M // P       # 16 m-tiles
HT = MLP // P     # 32 hidden tiles
NK = 512          # matmul free-dim chunk
MC = M // NK      # 4 context chunks
KNW = 65          # kn block width: 64 hd + 1 ones
EPS = 1e-5

_PROGRAM_CACHE = {}
_PHASE = int(os.environ.get("KPHASE", "5"))  # 1=QT 2=ctxT 3=x1 4=x2 5=full


def _build_program(trivial_affine, trivial_ffb):
    nc = bacc.Bacc(None)
    f32 = dt.float32
    bf = dt.bfloat16

    def din(name, shape, d=bf):
        return nc.declare_dram_parameter(name, list(shape), d, isOutput=False)

    xTq_d = din("xTq", [D, R])              # this core's columns of x^T
    xT_d = din("xT", [D, M])                # full batch x^T (for K1)
    encT_d = din("encT", [D, M])            # full batch enc^T (for K2)
    xrows_d = din("xrows", [R, D], f32)     # natural rows (residual)
    m01t_d = din("m01t", [P, MT], f32)      # 0 where masked, else 1 (tiled)
    m01b_d = din("m01b", [P, M])            # same, broadcast across partitions
    q1W_d = din("q1W", [D, D]); w1W_d = din("w1W", [D, D]); o1W_d = din("o1W", [D, D])
    q2W_d = din("q2W", [D, D]); w2W_d = din("w2W", [D, D]); o2W_d = din("o2W", [D, D])
    ffW1_d = din("ffW1", [D, MLP]); ffW2_d = din("ffW2", [MLP, D])
    ffb1h_d = din("ffb1h", [P, HT], f32)    # ffb1 tiled [P, ht]
    gb_d = {}
    if not trivial_affine:
        for nm in ("g1", "b1", "g2", "b2", "g3", "b3"):
            gb_d[nm] = din(nm + "b", [P, D], f32)
    if not trivial_ffb:
        ffb2b_d = din("ffb2b", [P, D], f32)
    out_d = nc.declare_dram_parameter("out", [R, D], f32, isOutput=True)

    lp = nc.allow_low_precision(reason="bf16 matmul staging")
    lp.__enter__()
    with tile.TileContext(nc) as tc:
        cpool = tc.alloc_tile_pool(name="const", bufs=1)
        small = tc.alloc_tile_pool(name="small", bufs=3)
        sbP = tc.alloc_tile_pool(name="sbP", bufs=1)

        ident_f = cpool.tile([P, P], f32)
        make_identity(nc, ident_f[:])
        ident_b = cpool.tile([P, P], bf)
        nc.vector.tensor_copy(ident_b[:], ident_f[:])
        ones_b = cpool.tile([1, HD], bf)
        nc.vector.memset(ones_b[:], 1.0)
        m01t_t = cpool.tile([P, MT], f32)
        nc.sync.dma_start(m01t_t[:], m01t_d[:])
        m01b_t = cpool.tile([P, M], bf)
        nc.sync.dma_start(m01b_t[:], m01b_d[:])
        if not trivial_ffb:
            ffb1h_t = cpool.tile([P, HT], f32)
            nc.sync.dma_start(ffb1h_t[:], ffb1h_d[:])

        # weight blocks: [P, DTL, D] bf16 (16KB/partition), 3-deep rotation
        def wblock(name, dram, sub=None):
            t = sbP.tile([P, DTL, D], bf, tag="W", bufs=3, name=name)
            if sub is None:
                nc.sync.dma_start(t[:], dram.rearrange("(dt p) c -> p dt c", p=P))
            else:
                nc.sync.dma_start(t[:], sub)
            return t

        # eviction engine alternation (PSUM fp32 -> SBUF bf16/f32)
        ev_par = [0]

        def evict(dst, src):
            if ev_par[0] % 2 == 0:
                nc.vector.tensor_copy(dst, src)
            else:
                nc.scalar.copy(dst, src)
            ev_par[0] += 1

        def layernorm(rsb, out_nat, gkey):
            """out_nat [P, D] = LN(rsb) * g + b.  Trashes rsb."""
            st = small.tile([P, 2, 6], f32, tag="ln_st")
            nc.vector.bn_stats(st[:, 0, :], rsb[:, 0:512])
            nc.vector.bn_stats(st[:, 1, :], rsb[:, 512:1024])
            mv = small.tile([P, 2], f32, tag="ln_mv")
            nc.vector.bn_aggr(mv[:], st[:])
            t = small.tile([P, 1], f32, tag="ln_t")
            nc.vector.tensor_scalar_add(t[:], mv[:, 1:2], EPS)
            s = small.tile([P, 1], f32, tag="ln_s")
            nc.scalar.sqrt(s[:], t[:])
            r0 = small.tile([P, 1], f32, tag="ln_r0")
            nc.vector.reciprocal(r0[:], s[:])
            # one Newton step: r1 = r0 * (1.5 - 0.5 * t * r0^2)
            u = small.tile([P, 1], f32, tag="ln_u")
            nc.vector.tensor_tensor(out=u[:], in0=t[:], in1=r0[:], op=ALU.mult)
            nc.vector.tensor_tensor(out=u[:], in0=u[:], in1=r0[:], op=ALU.mult)
            nc.vector.tensor_scalar(u[:], u[:], -0.5, 1.5, ALU.mult, ALU.add)
            r1 = small.tile([P, 1], f32, tag="ln_r1")
            nc.vector.tensor_tensor(out=r1[:], in0=r0[:], in1=u[:], op=ALU.mult)
            nc.vector.tensor_scalar(rsb[:], rsb[:], mv[:, 0:1], None, ALU.subtract)
            if trivial_affine:
                nc.vector.tensor_scalar(out_nat[:], rsb[:], r1[:], None, ALU.mult)
            else:
                g_t = small.tile([P, D], f32, tag="ln_g", bufs=2)
                nc.sync.dma_start(g_t[:], gb_d["g" + gkey][:])
                b_t = small.tile([P, D], f32, tag="ln_b", bufs=2)
                nc.sync.dma_start(b_t[:], gb_d["b" + gkey][:])
                nc.vector.tensor_scalar(rsb[:], rsb[:], r1[:], None, ALU.mult)
                nc.vector.tensor_tensor(out=rsb[:], in0=rsb[:], in1=g_t[:], op=ALU.mult)
                nc.vector.tensor_tensor(out=out_nat[:], in0=rsb[:], in1=b_t[:], op=ALU.add)

        xrows_t = sbP.tile([P, LT, D], f32, tag="xnat", bufs=2, name="xrows")
        nc.sync.dma_start(xrows_t[:], xrows_d.rearrange("(lt p) d -> p lt d", p=P))

        def qproj(wq, xqa, QT, pp):
            """QT[P, CT, R] bf16 = (x @ qW)^T for this core's rows."""
            for co in range(2):
                for ct in range(4):
                    ps = pp.tile([P, NK], f32, tag="pq", bufs=8, name="pq")
                    for dti in range(DTL):
                        nc.tensor.matmul(
                            ps[:], wq[:, dti, co * NK + ct * P:co * NK + ct * P + P],
                            xqa[:, dti, :],
                            start=(dti == 0), stop=(dti == DTL - 1))
                    evict(QT[:, co * 4 + ct, :], ps[:])

        def kproj_units(wk, kT_dram, sink, pp, pq_bufs):
            """K projection generator: yields after each 2-matmul unit.

            sink(g, mc, ps) consumes each finished [P, NK] PSUM group.
            With pq_bufs=8 all 8 groups of an mc are open at once (proj
            phase); with fewer bufs the group loop still works, just with
            tighter rotation.
            """
            src = kT_dram.rearrange("(dt p) m -> p dt m", p=P)
            for mc in range(MC):
                if pq_bufs >= 8:
                    pss = [pp.tile([P, NK], f32, tag="pq", bufs=pq_bufs,
                                   name=f"pk{g}") for g in range(8)]
                    for half in range(4):
                        xc = sbP.tile([P, 2, NK], bf, tag="xc", bufs=2, name="xc")
                        nc.sync.dma_start(
                            xc[:], src[:, 2 * half:2 * half + 2, bass.ts(mc, NK)])
                        for g in range(8):
                            for i2 in range(2):
                                co, ct = g // 4, g % 4
                                nc.tensor.matmul(
                                    pss[g][:],
                                    wk[:, 2 * half + i2,
                                       co * NK + ct * P:co * NK + ct * P + P],
                                    xc[:, i2, :],
                                    start=(half == 0 and i2 == 0),
                                    stop=(half == 3 and i2 == 1))
                            yield
                    for g in range(8):
                        sink(g, mc, pss[g])
                        yield
                else:
                    # group pairs with chunk reload: only 2 PSUM banks and
                    # one small moving tile live at a time (filler mode; the
                    # extra DMA re-reads ride the idle DMA engine)
                    for gp in range(4):
                        pss = [pp.tile([P, NK], f32, tag="pq", bufs=pq_bufs,
                                       name=f"pk{g}") for g in range(2)]
                        for half in range(4):
                            xc = sbP.tile([P, 2, NK], bf, tag="xc", bufs=2,
                                          name="xc")
                            nc.sync.dma_start(
                                xc[:],
                                src[:, 2 * half:2 * half + 2, bass.ts(mc, NK)])
                            for gi in range(2):
                                g = 2 * gp + gi
                                co, ct = g // 4, g % 4
                                for i2 in range(2):
                                    nc.tensor.matmul(
                                        pss[gi][:],
                                        wk[:, 2 * half + i2,
                                           co * NK + ct * P:co * NK + ct * P + P],
                                        xc[:, i2, :],
                                        start=(half == 0 and i2 == 0),
                                        stop=(half == 3 and i2 == 1))
                                yield
                        for gi in range(2):
                            sink(2 * gp + gi, mc, pss[gi])
                            yield

        def kproj(wk, kT_dram, KT, pp, masked=False):
            def sink(g, mc, ps):
                if masked:
                    # fold the encoder mask in: zero masked key columns
                    nc.vector.tensor_tensor(out=KT[:, g, bass.ts(mc, NK)],
                                            in0=ps[:],
                                            in1=m01b_t[:, bass.ts(mc, NK)],
                                            op=ALU.mult)
                else:
                    evict(KT[:, g, bass.ts(mc, NK)], ps[:])
            for _ in kproj_units(wk, kT_dram, sink, pp, 8):
                pass

        def dbg_out(src_ap):
            stg = sbP.tile([P, LT, D], f32, tag="dbg", name="dbg")
            nc.vector.tensor_copy(stg[:].rearrange("p a b -> p (a b)"), src_ap)
            nc.sync.dma_start(out_d.rearrange("(lt p) d -> p lt d", p=P), stg[:])

        def attention(qW_dram, wW_dram, oW_dram, q_src, kT_dram, use_mask,
                      resid_nat, x_out, gkey, want_xt, wq=None, wk=None,
                      prefetch=None, filler_factory=None, kt_src=None):
            """One MHA block + residual + LN.

            q_src: DRAM handle [D, R] or sbuf tile [P, DTL, R] bf16
            resid_nat/x_out: sbuf [P, LT, D] f32
            prefetch: callback emitted mid-core (weight DMA issue points)
            filler_factory(pa): generator of independent PE work interleaved
                into the attention core (keeps the PE P-state high)
            kt_src: DRAM scratch holding precomputed KT (skips kproj)
            returns xT_out sbuf [P, DTL, R] bf16 if want_xt
            """
            if wq is None:
                wq = wblock("wq", qW_dram)
            if wk is None and kt_src is None:
                wk = wblock("wk", wW_dram)

            if isinstance(q_src, bass.DRamTensorHandle):
                xqa = sbP.tile([P, DTL, NK], bf, tag="xT_fam", bufs=2, name="xqa")
                nc.sync.dma_start(xqa[:], q_src.rearrange("(dt p) r -> p dt r", p=P))
            else:
                xqa = q_src

            QT = sbP.tile([P, CT, R], bf, tag="QT", bufs=1, name="QT")
            KT = sbP.tile([P, CT, M], bf, tag="KT", bufs=1, name="KT")
            if kt_src is not None:
                nc.sync.dma_start(KT[:], kt_src[:])
            with tc.tile_pool(name="ps_proj", bufs=1, space="PSUM") as pp:
                qproj(wq, xqa, QT, pp)
                if kt_src is None:
                    kproj(wk, kT_dram, KT, pp, masked=use_mask)

            if _PHASE == 1:
                dbg_out(QT[:].rearrange("p a b -> p (a b)"))
                return None

            wo = wblock("wo", oW_dram)
            if prefetch is not None:
                prefetch()

            ctxT = sbP.tile([P, H // 2, R], bf, tag="ctxT", bufs=1, name="ctxT")

            def build_kn(hp, pa):
                kn = sbP.tile([P, MT, 2, KNW], bf, tag="kn", bufs=2, name="kn")
                if use_mask:
                    # ones column carries the mask so Z skips masked keys
                    for j in range(2):
                        nc.vector.tensor_copy(
                            kn[:, :, j, HD:KNW].rearrange("p a b -> p (a b)"),
                            m01t_t[:])
                else:
                    nc.vector.memset(kn[:, :, :, HD:KNW], 1.0)
                for mh in range(4):
                    tp = pa.tile([P, 4, P], bf, tag="knt", bufs=1, name="tp")
                    for j in range(4):
                        nc.tensor.transpose(
                            tp[:, j, :], KT[:, hp, bass.ts(4 * mh + j, P)],
                            ident_b[:])
                    nc.vector.tensor_copy(
                        kn[:, 4 * mh:4 * mh + 4, :, 0:HD],
                        tp[:].rearrange("p m (j h) -> p m j h", h=HD))
                return kn

            # --- attention core: head pairs, scores transposed ---
            s2_bufs = 1 if filler_factory is not None else 2
            with tc.tile_pool(name="ps_attn", bufs=1, space="PSUM") as pa:
                filler = filler_factory(pa) if filler_factory is not None else None
                fill_done = filler is None

                def fill(n):
                    nonlocal fill_done
                    if fill_done:
                        return
                    try:
                        for _ in range(n):
                            next(filler)
                    except StopIteration:
                        fill_done = True

                kn_cur = build_kn(0, pa)
                for hp in range(H // 2):
                    kn_next = build_kn(hp + 1, pa) if hp + 1 < H // 2 else None
                    ctxp = [pa.tile([P, NK], f32, tag="ctx", bufs=2, name=f"ctx{j}")
                            for j in range(2)]
                    for mt in range(MT):
                        s2 = pa.tile([P, 2, NK], f32, tag="s2", bufs=s2_bufs)
                        for j in range(2):
                            nc.tensor.matmul(
                                s2[:, j, :],
                                KT[bass.ts(j, HD), hp, bass.ts(mt, P)],
                                QT[bass.ts(j, HD), hp, :],
                                start=True, stop=True)
                        p2 = sbP.tile([P, 2, NK], bf, tag="p2", bufs=2, name="p2")
                        nc.scalar.activation(p2[:], s2[:], AF.Exp, scale=0.125)
                        fill(1)
                        for j in range(2):
                            nc.tensor.matmul(
                                ctxp[j][0:KNW, :],
                                kn_cur[:, mt, j, :], p2[:, j, :],
                                start=(mt == 0), stop=(mt == MT - 1))
                    # Evict ctx unnormalized (frees the banks fast), then
                    # broadcast Z via PE into a spare bank, reciprocal
                    # PSUM->bf16, and scale ctxT in place.
                    pz = pa.tile([P, NK], f32, tag="pz", bufs=1, name="pz")
                    for j in range(2):
                        dnb = small.tile([1, NK], bf, tag="dnb", bufs=2)
                        nc.vector.tensor_copy(dnb[:], ctxp[j][HD:HD + 1, :])
                        nc.vector.tensor_copy(ctxT[bass.ts(j, HD), hp, :],
                                              ctxp[j][0:HD, :])
                        nc.tensor.matmul(pz[bass.ts(j, HD), :], ones_b[:], dnb[:],
                                         start=True, stop=True)
                    zbr = small.tile([P, NK], bf, tag="zbr", bufs=2)
                    nc.vector.reciprocal(zbr[:], pz[:])
                    for j in range(2):
                        nc.vector.tensor_tensor(out=ctxT[bass.ts(j, HD), hp, :],
                                                in0=ctxT[bass.ts(j, HD), hp, :],
                                                in1=zbr[bass.ts(j, HD), :],
                                                op=ALU.mult)
                    kn_cur = kn_next
                fill(1000)
            if _PHASE == 2:
                dbg_out(ctxT[:].rearrange("p a b -> p (a b)"))
                return None

            # --- output projection (full 128-contract) + residual + LN ---
            xT_out = None
            if want_xt:
                xT_out = sbP.tile([P, DTL, R], bf, tag="xT_fam", bufs=2,
                                  name="xT_out")
            with tc.tile_pool(name="ps_o", bufs=1, space="PSUM") as po:
                for lt in range(LT):
                    rsb = small.tile([P, D], f32, tag="rsb", bufs=1)
                    for dc in range(2):
                        ps = po.tile([P, NK], f32, tag="po", bufs=4, name="ps_o")
                        for i in range(H // 2):
                            nc.tensor.matmul(
                                ps[:],
                                ctxT[:, i, bass.ts(lt, P)],
                                wo[:, i, bass.ts(dc, NK)],
                                start=(i == 0), stop=(i == H // 2 - 1))
                        nc.vector.tensor_tensor(out=rsb[:, bass.ts(dc, NK)],
                                                in0=ps[:],
                                                in1=resid_nat[:, lt, bass.ts(dc, NK)],
                                                op=ALU.add)
                    layernorm(rsb, x_out[:, lt, :], gkey)
                # transposes emitted after all LNs so the PE never waits on
                # a just-computed LN result
                if want_xt:
                    for lt in range(LT):
                        for half in range(2):
                            tp = po.tile([P, 4, P], f32, tag="pt", bufs=2)
                            for i in range(4):
                                nc.tensor.transpose(
                                    tp[:, i, :],
                                    x_out[:, lt, bass.ts(4 * half + i, P)],
                                    ident_f[:])
                            nc.vector.tensor_copy(
                                xT_out[:, 4 * half:4 * half + 4, bass.ts(lt, P)],
                                tp[:])
            return xT_out

        # ---------------- program ----------------
        dpool = tc.alloc_tile_pool(name="dscr", bufs=1, space="DRAM")
        kt2scr = dpool.tile([P, CT, M], bf, tag="kt2", bufs=1, name="kt2scr")

        wq1 = wblock("wq1", q1W_d)
        wk1 = wblock("wk1", w1W_d)

        pf_state = {}

        def prefetch1():
            pf_state["wq2"] = wblock("wq2", q2W_d)
            pf_state["wk2"] = wblock("wk2", w2W_d)

        def k2_sink(g, mc, ps):
            stg = sbP.tile([P, NK], bf, tag="kstg", bufs=2, name="kstg")
            nc.vector.tensor_copy(stg[:], ps[:])
            nc.sync.dma_start(kt2scr[:, g, bass.ts(mc, NK)], stg[:])

        def k2_filler(pa):
            return kproj_units(pf_state["wk2"], encT_d, k2_sink, pa, 2)

        x1 = sbP.tile([P, LT, D], f32, tag="xnat", bufs=2, name="x1")
        x1T = attention(q1W_d, w1W_d, o1W_d, xTq_d, xT_d, False,
                        xrows_t, x1, "1", want_xt=(_PHASE > 3),
                        wq=wq1, wk=wk1, prefetch=prefetch1)
        if _PHASE == 3:
            nc.sync.dma_start(out_d.rearrange("(lt p) d -> p lt d", p=P), x1[:])

        if _PHASE >= 4:
            f1b = {}

            def prefetch2():
                f1b[0] = wblock("f10", ffW1_d,
                                ffW1_d.rearrange("(dt p) h -> p dt h", p=P)[:, :, 0:D])
                f1b[1] = wblock("f11", ffW1_d,
                                ffW1_d.rearrange("(dt p) h -> p dt h", p=P)[:, :, D:2 * D])

            x2 = sbP.tile([P, LT, D], f32, tag="xnat", bufs=2, name="x2")
            x2T = attention(q2W_d, w2W_d, o2W_d, x1T, encT_d, True,
                            x1, x2, "2", want_xt=(_PHASE > 4),
                            wq=pf_state["wq2"], wk=pf_state["wk2"],
                            prefetch=prefetch2)
            if _PHASE == 4:
                nc.sync.dma_start(out_d.rearrange("(lt p) d -> p lt d", p=P), x2[:])

        # --- FFN ---
        if _PHASE >= 5:
            ff1r = ffW1_d.rearrange("(dt p) h -> p dt h", p=P)
            ff2r = ffW2_d.rearrange("(ht p) d -> p ht d", p=P)
            x3 = sbP.tile([P, LT, D], f32, tag="xnat", bufs=2, name="x3")
            hT = sbP.tile([P, HT, R], bf, tag="hT", bufs=1, name="hT")
            f2b = {}
            with tc.tile_pool(name="ps_ff1", bufs=1, space="PSUM") as pf1:
                for q in range(4):
                    if q >= 2:
                        f1b[q] = wblock(f"f1{q}", ffW1_d,
                                        ff1r[:, :, q * D:(q + 1) * D])
                    if q == 3:
                        f2b[0] = wblock("f20", ffW2_d, ff2r[:, 0:DTL, :])
                    wt = f1b[q]
                    for hc in range(DTL):
                        ht = q * DTL + hc
                        ps = pf1.tile([P, R], f32, tag="ph", bufs=2)
                        for dti in range(DTL):
                            nc.tensor.matmul(ps[:], wt[:, dti, bass.ts(hc, P)],
                                             x2T[:, dti, :],
                                             start=(dti == 0), stop=(dti == DTL - 1))
                        bias = 0.0 if trivial_ffb else ffb1h_t[:, ht:ht + 1]
                        nc.scalar.activation(hT[:, ht, :], ps[:], AF.Gelu,
                                             bias=bias, scale=1.0)

            with tc.tile_pool(name="ps_ff2", bufs=1, space="PSUM") as pf2:
                psl = [pf2.tile([P, 2, NK], f32, tag=f"pf{lt}", bufs=1,
                                name=f"pf{lt}") for lt in range(LT)]
                for q in range(4):
                    if q < 3:
                        # stream next ffW2 block; buffer freed by earlier reads
                        f2b[q + 1] = wblock(f"f2{q + 1}", ffW2_d,
                                            ff2r[:, (q + 1) * DTL:(q + 2) * DTL, :])
                    wt = f2b[q]
                    for hc in range(DTL):
                        ht = q * DTL + hc
                        for lt in range(LT):
                            for dc in range(2):
                                nc.tensor.matmul(
                                    psl[lt][:, dc, :],
                                    hT[:, ht, bass.ts(lt, P)],
                                    wt[:, hc, bass.ts(dc, NK)],
                                    start=(ht == 0), stop=(ht == HT - 1))
                for lt in range(LT):
                    rsb3 = small.tile([P, D], f32, tag="rsb", bufs=1)
                    for dc in range(2):
                        if trivial_ffb:
                            nc.vector.tensor_tensor(out=rsb3[:, bass.ts(dc, NK)],
                                                    in0=psl[lt][:, dc, :],
                                                    in1=x2[:, lt, bass.ts(dc, NK)],
                                                    op=ALU.add)
                        else:
                            ffb2c = small.tile([P, NK], f32, tag="ffb2", bufs=2)
                            nc.sync.dma_start(ffb2c[:], ffb2b_d[:, bass.ts(dc, NK)])
                            nc.vector.tensor_tensor(out=rsb3[:, bass.ts(dc, NK)],
                                                    in0=psl[lt][:, dc, :],
                                                    in1=ffb2c[:],
                                                    op=ALU.add)
                            nc.vector.tensor_tensor(out=rsb3[:, bass.ts(dc, NK)],
                                                    in0=rsb3[:, bass.ts(dc, NK)],
                                                    in1=x2[:, lt, bass.ts(dc, NK)],
                                                    op=ALU.add)
                    layernorm(rsb3, x3[:, lt, :], "3")
            nc.sync.dma_start(out_d.rearrange("(lt p) d -> p lt d", p=P), x3[:])

        sbP.release()
        small.release()
        cpool.release()

    lp.__exit__(None, None, None)
    nc.compile()
    return nc


def _host_prep(inputs):
    bfd = ml_dtypes.bfloat16
    x = np.ascontiguousarray(np.asarray(inputs["x"], np.float32))
    enc = np.ascontiguousarray(np.asarray(inputs["enc_output"], np.float32))
    mask = np.asarray(inputs["mask"])

    n = np.arange(D) // HD
    d = np.arange(D) % HD
    perm = d * H + n

    def pw(q, w, o):
        return (np.ascontiguousarray(np.asarray(q, np.float32)[:, perm].astype(bfd)),
                np.ascontiguousarray(np.asarray(w, np.float32)[:, perm].astype(bfd)),
                np.ascontiguousarray(np.asarray(o, np.float32)[perm, :].astype(bfd)))

    q1W, w1W, o1W = pw(inputs["q1W"], inputs["w1W"], inputs["o1W"])
    q2W, w2W, o2W = pw(inputs["q2W"], inputs["w2W"], inputs["o2W"])
    ffW1 = np.ascontiguousarray(np.asarray(inputs["ffW1"], np.float32).astype(bfd))
    ffW2 = np.ascontiguousarray(np.asarray(inputs["ffW2"], np.float32).astype(bfd))
    ffb1 = np.asarray(inputs["ffb1"], np.float32)
    ffb2 = np.asarray(inputs["ffb2"], np.float32)
    g = {k: np.asarray(inputs[k], np.float32)
         for k in ("g1", "b1", "g2", "b2", "g3", "b3")}

    trivial_affine = all(
        np.all(g[f"g{i}"] == 1.0) and np.all(g[f"b{i}"] == 0.0) for i in (1, 2, 3))
    trivial_ffb = bool(np.all(ffb1 == 0.0) and np.all(ffb2 == 0.0))

    mask01 = np.where(mask[:, 0, :, 0], np.float32(0.0),
                      np.float32(1.0)).astype(np.float32)

    xT = [np.ascontiguousarray(x[b].T.astype(bfd)) for b in range(B)]
    encT = [np.ascontiguousarray(enc[b].T.astype(bfd)) for b in range(B)]
    m01t = [np.ascontiguousarray(mask01[b].reshape(MT, P).T) for b in range(B)]
    m01b = [np.ascontiguousarray(
        np.broadcast_to(mask01[b].astype(bfd), (P, M))) for b in range(B)]

    in_maps = []
    for c in range(NCORES):
        b, grp = c // GROUPS, c % GROUPS
        im = {
            "xTq": np.ascontiguousarray(xT[b][:, grp * R:(grp + 1) * R]),
            "xT": xT[b],
            "encT": encT[b],
            "xrows": np.ascontiguousarray(x[b, grp * R:(grp + 1) * R, :]),
            "m01t": m01t[b], "m01b": m01b[b],
            "q1W": q1W, "w1W": w1W, "o1W": o1W,
            "q2W": q2W, "w2W": w2W, "o2W": o2W,
            "ffW1": ffW1, "ffW2": ffW2,
            "ffb1h": np.ascontiguousarray(ffb1.reshape(HT, P).T),
        }
        if not trivial_affine:
            for k in ("g1", "b1", "g2", "b2", "g3", "b3"):
                im[k + "b"] = np.ascontiguousarray(
                    np.broadcast_to(g[k], (P, D)).astype(np.float32))
        if not trivial_ffb:
            im["ffb2b"] = np.ascontiguousarray(
                np.broadcast_to(ffb2, (P, D)).astype(np.float32))
        in_maps.append(im)
    return in_maps, trivial_affine, trivial_ffb


def kernel(**inputs) -> np.ndarray:
    in_maps, trivial_affine, trivial_ffb = _host_prep(inputs)
    key = (trivial_affine, trivial_ffb)
    if key not in _PROGRAM_CACHE:
        _PROGRAM_CACHE[key] = _build_program(*key)
    nc = _PROGRAM_CACHE[key]
    res = run_bass_kernel_spmd(nc, in_maps, list(range(NCORES)))
    out = np.empty((B, L, D), np.float32)
    for c in range(NCORES):
        b, grp = c // GROUPS, c % GROUPS
        out[b, grp * R:(grp + 1) * R, :] = res.results[c]["out"]
    return out


# revision 39
# speedup vs baseline: 1.3563x; 1.0098x over previous
"""Trainium2 Bass kernel for a transformer decoder layer (self-attn + cross-attn + FFN).

Sharding: 8 cores, data-parallel over (batch, seq): core c handles batch c//4,
rows (c%4)*512:(c%4+1)*512. No collectives; the K projections (which need the
full 2048-token context) are computed replicated per core.

v2 design notes:
  - all matmul operands bf16 (weights cast host-side); PSUM/LN/residual fp32.
  - weights DMA'd as whole [1024,1024]-sized blocks through a 3-deep rotating
    pool tag, prefetched one phase ahead (ffW1/ffW2 stream as 4 blocks each).
  - attention scores transposed s^T[m(part), l(free)]; encoder mask folded as
    per-partition bias into the Exp activation; softmax denominator comes from
    a ones-column appended to K-natural (66-wide) in the value matmul, then
    Z is broadcast via a 1x64 ones matmul and applied with a DVE divide.
  - output projection contracts the full 128-partition head-pair tile.
  - FFN gelu runs as a single scalar-engine Gelu activation per h-tile.
"""

import os
import sys

sys.path.insert(0, "/opt/trn_rl_repo")

import numpy as np
import ml_dtypes

import concourse.bass as bass
import concourse.bacc as bacc
import concourse.mybir as mybir
import concourse.tile as tile
from concourse.bass_utils import run_bass_kernel_spmd
from concourse.masks import make_identity

dt = mybir.dt
AF = mybir.ActivationFunctionType
ALU = mybir.AluOpType

P = 128
D = 1024          # d_model
H = 16            # heads
HD = 64           # head dim
MLP = 4096
B, L, M = 2, 2048, 2048
NCORES = 8
GROUPS = 4        # cores per batch
R = L // GROUPS   # 512 rows per core
LT = R // P       # 4 l-tiles per core
DTL = D // P      # 8 d-tiles
CT = D // P       # 8 c-tiles
MT = M // P       # 16 m-tiles
HT = MLP // P     # 32 hidden tiles
NK = 512          # matmul free-dim chunk
MC = M // NK      # 4 context chunks
KNW = 65          # kn block width: 64 hd + 1 ones
EPS = 1e-5

_PROGRAM_CACHE = {}
_PHASE = int(os.environ.get("KPHASE", "5"))  # 1=QT 2=ctxT 3=x1 4=x2 5=full


def _build_program(trivial_affine, trivial_ffb):
    nc = bacc.Bacc(None)
    f32 = dt.float32
    bf = dt.bfloat16

    def din(name, shape, d=bf):
        return nc.declare_dram_parameter(name, list(shape), d, isOutput=False)

    xTq_d = din("xTq", [D, R])              # this core's columns of x^T
    xT_d = din("xT", [D, M])                # full batch x^T (for K1)
    encT_d = din("encT", [D, M])            # full batch enc^T (for K2)
    xrows_d = din("xrows", [R, D], f32)     # natural rows (residual)
    m01t_d = din("m01t", [P, MT], f32)      # 0 where masked, else 1 (tiled)
    m01b_d = din("m01b", [P, M])            # same, broadcast across partitions
    q1W_d = din("q1W", [D, D]); w1W_d = din("w1W", [D, D]); o1W_d = din("o1W", [D, D])
    q2W_d = din("q2W", [D, D]); w2W_d = din("w2W", [D, D]); o2W_d = din("o2W", [D, D])
    ffW1_d = din("ffW1", [D, MLP]); ffW2_d = din("ffW2", [MLP, D])
    ffb1h_d = din("ffb1h", [P, HT], f32)    # ffb1 tiled [P, ht]
    gb_d = {}
    if not trivial_affine:
        for nm in ("g1", "b1", "g2", "b2", "g3", "b3"):
            gb_d[nm] = din(nm + "b", [P, D], f32)
    if not trivial_ffb:
        ffb2b_d = din("ffb2b", [P, D], f32)
    out_d = nc.declare_dram_parameter("out", [R, D], f32, isOutput=True)

    lp = nc.allow_low_precision(reason="bf16 matmul staging")
    lp.__enter__()
    with tile.TileContext(nc) as tc:
        cpool = tc.alloc_tile_pool(name="const", bufs=1)
        small = tc.alloc_tile_pool(name="small", bufs=3)
        sbP = tc.alloc_tile_pool(name="sbP", bufs=1)

        ident_f = cpool.tile([P, P], f32)
        make_identity(nc, ident_f[:])
        ident_b = cpool.tile([P, P], bf)
        nc.vector.tensor_copy(ident_b[:], ident_f[:])
        ones_b = cpool.tile([1, HD], bf)
        nc.vector.memset(ones_b[:], 1.0)
        m01t_t = cpool.tile([P, MT], f32)
        nc.sync.dma_start(m01t_t[:], m01t_d[:])
        m01b_t = cpool.tile([P, M], bf)
        nc.sync.dma_start(m01b_t[:], m01b_d[:])
        if not trivial_ffb:
            ffb1h_t = cpool.tile([P, HT], f32)
            nc.sync.dma_start(ffb1h_t[:], ffb1h_d[:])

        # weight blocks: [P, DTL, D] bf16 (16KB/partition), 3-deep rotation
        def wblock(name, dram, sub=None):
            t = sbP.tile([P, DTL, D], bf, tag="W", bufs=3, name=name)
            if sub is None:
                nc.sync.dma_start(t[:], dram.rearrange("(dt p) c -> p dt c", p=P))
            else:
                nc.sync.dma_start(t[:], sub)
            return t

        # eviction engine alternation (PSUM fp32 -> SBUF bf16/f32)
        ev_par = [0]

        def evict(dst, src):
            if ev_par[0] % 2 == 0:
                nc.vector.tensor_copy(dst, src)
            else:
                nc.scalar.copy(dst, src)
            ev_par[0] += 1

        def layernorm(rsb, out_nat, gkey):
            """out_nat [P, D] = LN(rsb) * g + b.  Trashes rsb."""
            st = small.tile([P, 2, 6], f32, tag="ln_st")
            nc.vector.bn_stats(st[:, 0, :], rsb[:, 0:512])
            nc.vector.bn_stats(st[:, 1, :], rsb[:, 512:1024])
            mv = small.tile([P, 2], f32, tag="ln_mv")
            nc.vector.bn_aggr(mv[:], st[:])
            t = small.tile([P, 1], f32, tag="ln_t")
            nc.vector.tensor_scalar_add(t[:], mv[:, 1:2], EPS)
            s = small.tile([P, 1], f32, tag="ln_s")
            nc.scalar.sqrt(s[:], t[:])
            r0 = small.tile([P, 1], f32, tag="ln_r0")
            nc.vector.reciprocal(r0[:], s[:])
            # one Newton step: r1 = r0 * (1.5 - 0.5 * t * r0^2)
            u = small.tile([P, 1], f32, tag="ln_u")
            nc.vector.tensor_tensor(out=u[:], in0=t[:], in1=r0[:], op=ALU.mult)
            nc.vector.tensor_tensor(out=u[:], in0=u[:], in1=r0[:], op=ALU.mult)
            nc.vector.tensor_scalar(u[:], u[:], -0.5, 1.5, ALU.mult, ALU.add)
            r1 = small.tile([P, 1], f32, tag="ln_r1")
            nc.vector.tensor_tensor(out=r1[:], in0=r0[:], in1=u[:], op=ALU.mult)
            nc.vector.tensor_scalar(rsb[:], rsb[:], mv[:, 0:1], None, ALU.subtract)
            if trivial_affine:
                nc.vector.tensor_scalar(out_nat[:], rsb[:], r1[:], None, ALU.mult)
            else:
                g_t = small.tile([P, D], f32, tag="ln_g", bufs=2)
                nc.sync.dma_start(g_t[:], gb_d["g" + gkey][:])
                b_t = small.tile([P, D], f32, tag="ln_b", bufs=2)
                nc.sync.dma_start(b_t[:], gb_d["b" + gkey][:])
                nc.vector.tensor_scalar(rsb[:], rsb[:], r1[:], None, ALU.mult)
                nc.vector.tensor_tensor(out=rsb[:], in0=rsb[:], in1=g_t[:], op=ALU.mult)
                nc.vector.tensor_tensor(out=out_nat[:], in0=rsb[:], in1=b_t[:], op=ALU.add)

        xrows_t = sbP.tile([P, LT, D], f32, tag="xnat", bufs=2, name="xrows")
        nc.sync.dma_start(xrows_t[:], xrows_d.rearrange("(lt p) d -> p lt d", p=P))

        def qproj(wq, xqa, QT, pp):
            """QT[P, CT, R] bf16 = (x @ qW)^T for this core's rows."""
            for co in range(2):
                for ct in range(4):
                    ps = pp.tile([P, NK], f32, tag="pq", bufs=8, name="pq")
                    for dti in range(DTL):
                        nc.tensor.matmul(
                            ps[:], wq[:, dti, co * NK + ct * P:co * NK + ct * P + P],
                            xqa[:, dti, :],
                            start=(dti == 0), stop=(dti == DTL - 1))
                    evict(QT[:, co * 4 + ct, :], ps[:])

        def kproj_units(wk, kT_dram, sink, pp, pq_bufs):
            """K projection generator: yields after each 2-matmul unit.

            sink(g, mc, ps) consumes each finished [P, NK] PSUM group.
            With pq_bufs=8 all 8 groups of an mc are open at once (proj
            phase); with fewer bufs the group loop still works, just with
            tighter rotation.
            """
            src = kT_dram.rearrange("(dt p) m -> p dt m", p=P)
            for mc in range(MC):
                if pq_bufs >= 8:
                    pss = [pp.tile([P, NK], f32, tag="pq", bufs=pq_bufs,
                                   name=f"pk{g}") for g in range(8)]
                    for half in range(4):
                        xc = sbP.tile([P, 2, NK], bf, tag="xc", bufs=2, name="xc")
                        nc.sync.dma_start(
                            xc[:], src[:, 2 * half:2 * half + 2, bass.ts(mc, NK)])
                        for g in range(8):
                            for i2 in range(2):
                                co, ct = g // 4, g % 4
                                nc.tensor.matmul(
                                    pss[g][:],
                                    wk[:, 2 * half + i2,
                                       co * NK + ct * P:co * NK + ct * P + P],
                                    xc[:, i2, :],
                                    start=(half == 0 and i2 == 0),
                                    stop=(half == 3 and i2 == 1))
                            yield
                    for g in range(8):
                        sink(g, mc, pss[g])
                        yield
                else:
                    # group pairs with chunk reload: only 2 PSUM banks and
                    # one small moving tile live at a time (filler mode; the
                    # extra DMA re-reads ride the idle DMA engine)
                    for gp in range(4):
                        pss = [pp.tile([P, NK], f32, tag="pq", bufs=pq_bufs,
                                       name=f"pk{g}") for g in range(2)]
                        for half in range(4):
                            xc = sbP.tile([P, 2, NK], bf, tag="xc", bufs=2,
                                          name="xc")
                            nc.sync.dma_start(
                                xc[:],
                                src[:, 2 * half:2 * half + 2, bass.ts(mc, NK)])
                            for gi in range(2):
                                g = 2 * gp + gi
                                co, ct = g // 4, g % 4
                                for i2 in range(2):
                                    nc.tensor.matmul(
                                        pss[gi][:],
                                        wk[:, 2 * half + i2,
                                           co * NK + ct * P:co * NK + ct * P + P],
                                        xc[:, i2, :],
                                        start=(half == 0 and i2 == 0),
                                        stop=(half == 3 and i2 == 1))
                                yield
                        for gi in range(2):
                            sink(2 * gp + gi, mc, pss[gi])
                            yield

        def kproj(wk, kT_dram, KT, pp, masked=False):
            def sink(g, mc, ps):
                if masked:
                    # fold the encoder mask in: zero masked key columns
                    nc.vector.tensor_tensor(out=KT[:, g, bass.ts(mc, NK)],
                                            in0=ps[:],
                                            in1=m01b_t[:, bass.ts(mc, NK)],
                                            op=ALU.mult)
                else:
                    evict(KT[:, g, bass.ts(mc, NK)], ps[:])
            for _ in kproj_units(wk, kT_dram, sink, pp, 8):
                pass

        def dbg_out(src_ap):
            stg = sbP.tile([P, LT, D], f32, tag="dbg", name="dbg")
            nc.vector.tensor_copy(stg[:].rearrange("p a b -> p (a b)"), src_ap)
            nc.sync.dma_start(out_d.rearrange("(lt p) d -> p lt d", p=P), stg[:])

        def attention(qW_dram, wW_dram, oW_dram, q_src, kT_dram, use_mask,
                      resid_nat, x_out, gkey, want_xt, wq=None, wk=None,
                      prefetch=None, filler_factory=None, kt_src=None):
            """One MHA block + residual + LN.

            q_src: DRAM handle [D, R] or sbuf tile [P, DTL, R] bf16
            resid_nat/x_out: sbuf [P, LT, D] f32
            prefetch: callback emitted mid-core (weight DMA issue points)
            filler_factory(pa): generator of independent PE work interleaved
                into the attention core (keeps the PE P-state high)
            kt_src: DRAM scratch holding precomputed KT (skips kproj)
            returns xT_out sbuf [P, DTL, R] bf16 if want_xt
            """
            if wq is None:
                wq = wblock("wq", qW_dram)
            if wk is None and kt_src is None:
                wk = wblock("wk", wW_dram)

            if isinstance(q_src, bass.DRamTensorHandle):
                xqa = sbP.tile([P, DTL, NK], bf, tag="xT_fam", bufs=2, name="xqa")
                nc.sync.dma_start(xqa[:], q_src.rearrange("(dt p) r -> p dt r", p=P))
            else:
                xqa = q_src

            QT = sbP.tile([P, CT, R], bf, tag="QT", bufs=1, name="QT")
            KT = sbP.tile([P, CT, M], bf, tag="KT", bufs=1, name="KT")
            if kt_src is not None:
                nc.sync.dma_start(KT[:], kt_src[:])
            with tc.tile_pool(name="ps_proj", bufs=1, space="PSUM") as pp:
                qproj(wq, xqa, QT, pp)
                if kt_src is None:
                    kproj(wk, kT_dram, KT, pp, masked=use_mask)

            if _PHASE == 1:
                dbg_out(QT[:].rearrange("p a b -> p (a b)"))
                return None

            wo = wblock("wo", oW_dram)
            if prefetch is not None:
                prefetch()

            ctxT = sbP.tile([P, H // 2, R], bf, tag="ctxT", bufs=1, name="ctxT")

            def build_kn(hp, pa):
                kn = sbP.tile([P, MT, 2, KNW], bf, tag="kn", bufs=2, name="kn")
                if use_mask:
                    # ones column carries the mask so Z skips masked keys
                    for j in range(2):
                        nc.vector.tensor_copy(
                            kn[:, :, j, HD:KNW].rearrange("p a b -> p (a b)"),
                            m01t_t[:])
                else:
                    nc.vector.memset(kn[:, :, :, HD:KNW], 1.0)
                for mh in range(4):
                    tp = pa.tile([P, 4, P], bf, tag="knt", bufs=1, name="tp")
                    for j in range(4):
                        nc.tensor.transpose(
                            tp[:, j, :], KT[:, hp, bass.ts(4 * mh + j, P)],
                            ident_b[:])
                    nc.vector.tensor_copy(
                        kn[:, 4 * mh:4 * mh + 4, :, 0:HD],
                        tp[:].rearrange("p m (j h) -> p m j h", h=HD))
                return kn

            # --- attention core: head pairs, scores transposed ---
            s2_bufs = 1 if filler_factory is not None else 2
            with tc.tile_pool(name="ps_attn", bufs=1, space="PSUM") as pa:
                filler = filler_factory(pa) if filler_factory is not None else None
                fill_done = filler is None

                def fill(n):
                    nonlocal fill_done
                    if fill_done:
                        return
                    try:
                        for _ in range(n):
                            next(filler)
                    except StopIteration:
                        fill_done = True

                def normalize(hp, dnbs):
                    # broadcast Z via PE, reciprocal PSUM->bf16, scale in place
                    pz = pa.tile([P, NK], f32, tag="pz", bufs=1, name="pz")
                    for j in range(2):
                        nc.tensor.matmul(pz[bass.ts(j, HD), :], ones_b[:],
                                         dnbs[j][:], start=True, stop=True)
                    zbr = small.tile([P, NK], bf, tag="zbr", bufs=2)
                    nc.vector.reciprocal(zbr[:], pz[:])
                    for j in range(2):
                        nc.vector.tensor_tensor(out=ctxT[bass.ts(j, HD), hp, :],
                                                in0=ctxT[bass.ts(j, HD), hp, :],
                                                in1=zbr[bass.ts(j, HD), :],
                                                op=ALU.mult)

                pending = None
                kn_cur = build_kn(0, pa)
                for hp in range(H // 2):
                    kn_next = build_kn(hp + 1, pa) if hp + 1 < H // 2 else None
                    ctxp = [pa.tile([P, NK], f32, tag="ctx", bufs=2, name=f"ctx{j}")
                            for j in range(2)]
                    for mt in range(MT):
                        s2 = pa.tile([P, 2, NK], f32, tag="s2", bufs=s2_bufs)
                        for j in range(2):
                            nc.tensor.matmul(
                                s2[:, j, :],
                                KT[bass.ts(j, HD), hp, bass.ts(mt, P)],
                                QT[bass.ts(j, HD), hp, :],
                                start=True, stop=True)
                        p2 = sbP.tile([P, 2, NK], bf, tag="p2", bufs=2, name="p2")
                        nc.scalar.activation(p2[:], s2[:], AF.Exp, scale=0.125)
                        fill(1)
                        for j in range(2):
                            nc.tensor.matmul(
                                ctxp[j][0:KNW, :],
                                kn_cur[:, mt, j, :], p2[:, j, :],
                                start=(mt == 0), stop=(mt == MT - 1))
                        if mt == 7 and pending is not None:
                            # previous hp's normalize, emitted mid-loop where
                            # every input is long since ready
                            normalize(*pending)
                            pending = None
                    # evict ctx unnormalized (frees the banks fast)
                    dnbs = []
                    for j in range(2):
                        dnb = small.tile([1, NK], bf, tag="dnb", bufs=4)
                        nc.vector.tensor_copy(dnb[:], ctxp[j][HD:HD + 1, :])
                        nc.vector.tensor_copy(ctxT[bass.ts(j, HD), hp, :],
                                              ctxp[j][0:HD, :])
                        dnbs.append(dnb)
                    pending = (hp, dnbs)
                    kn_cur = kn_next
                normalize(*pending)
                fill(1000)
            if _PHASE == 2:
                dbg_out(ctxT[:].rearrange("p a b -> p (a b)"))
                return None

            # --- output projection (full 128-contract) + residual + LN ---
            xT_out = None
            if want_xt:
                xT_out = sbP.tile([P, DTL, R], bf, tag="xT_fam", bufs=2,
                                  name="xT_out")
            with tc.tile_pool(name="ps_o", bufs=1, space="PSUM") as po:
                for lt in range(LT):
                    rsb = small.tile([P, D], f32, tag="rsb", bufs=1)
                    for dc in range(2):
                        ps = po.tile([P, NK], f32, tag="po", bufs=4, name="ps_o")
                        for i in range(H // 2):
                            nc.tensor.matmul(
                                ps[:],
                                ctxT[:, i, bass.ts(lt, P)],
                                wo[:, i, bass.ts(dc, NK)],
                                start=(i == 0), stop=(i == H // 2 - 1))
                        nc.vector.tensor_tensor(out=rsb[:, bass.ts(dc, NK)],
                                                in0=ps[:],
                                                in1=resid_nat[:, lt, bass.ts(dc, NK)],
                                                op=ALU.add)
                    layernorm(rsb, x_out[:, lt, :], gkey)
                # transposes emitted after all LNs so the PE never waits on
                # a just-computed LN result
                if want_xt:
                    for lt in range(LT):
                        for half in range(2):
                            tp = po.tile([P, 4, P], f32, tag="pt", bufs=2)
                            for i in range(4):
                                nc.tensor.transpose(
                                    tp[:, i, :],
                                    x_out[:, lt, bass.ts(4 * half + i, P)],
                                    ident_f[:])
                            nc.vector.tensor_copy(
                                xT_out[:, 4 * half:4 * half + 4, bass.ts(lt, P)],
                                tp[:])
            return xT_out

        # ---------------- program ----------------
        dpool = tc.alloc_tile_pool(name="dscr", bufs=1, space="DRAM")
        kt2scr = dpool.tile([P, CT, M], bf, tag="kt2", bufs=1, name="kt2scr")

        wq1 = wblock("wq1", q1W_d)
        wk1 = wblock("wk1", w1W_d)

        pf_state = {}

        def prefetch1():
            pf_state["wq2"] = wblock("wq2", q2W_d)
            pf_state["wk2"] = wblock("wk2", w2W_d)

        def k2_sink(g, mc, ps):
            stg = sbP.tile([P, NK], bf, tag="kstg", bufs=2, name="kstg")
            nc.vector.tensor_copy(stg[:], ps[:])
            nc.sync.dma_start(kt2scr[:, g, bass.ts(mc, NK)], stg[:])

        def k2_filler(pa):
            return kproj_units(pf_state["wk2"], encT_d, k2_sink, pa, 2)

        x1 = sbP.tile([P, LT, D], f32, tag="xnat", bufs=2, name="x1")
        x1T = attention(q1W_d, w1W_d, o1W_d, xTq_d, xT_d, False,
                        xrows_t, x1, "1", want_xt=(_PHASE > 3),
                        wq=wq1, wk=wk1, prefetch=prefetch1)
        if _PHASE == 3:
            nc.sync.dma_start(out_d.rearrange("(lt p) d -> p lt d", p=P), x1[:])

        if _PHASE >= 4:
            f1b = {}

            def prefetch2():
                f1b[0] = wblock("f10", ffW1_d,
                                ffW1_d.rearrange("(dt p) h -> p dt h", p=P)[:, :, 0:D])
                f1b[1] = wblock("f11", ffW1_d,
                                ffW1_d.rearrange("(dt p) h -> p dt h", p=P)[:, :, D:2 * D])

            x2 = sbP.tile([P, LT, D], f32, tag="xnat", bufs=2, name="x2")
            x2T = attention(q2W_d, w2W_d, o2W_d, x1T, encT_d, True,
                            x1, x2, "2", want_xt=(_PHASE > 4),
                            wq=pf_state["wq2"], wk=pf_state["wk2"],
                            prefetch=prefetch2)
            if _PHASE == 4:
                nc.sync.dma_start(out_d.rearrange("(lt p) d -> p lt d", p=P), x2[:])

        # --- FFN ---
        if _PHASE >= 5:
            ff1r = ffW1_d.rearrange("(dt p) h -> p dt h", p=P)
            ff2r = ffW2_d.rearrange("(ht p) d -> p ht d", p=P)
            x3 = sbP.tile([P, LT, D], f32, tag="xnat", bufs=2, name="x3")
            hT = sbP.tile([P, HT, R], bf, tag="hT", bufs=1, name="hT")
            f2b = {}
            with tc.tile_pool(name="ps_ff1", bufs=1, space="PSUM") as pf1:
                for q in range(4):
                    if q >= 2:
                        f1b[q] = wblock(f"f1{q}", ffW1_d,
                                        ff1r[:, :, q * D:(q + 1) * D])
                    if q == 3:
                        f2b[0] = wblock("f20", ffW2_d, ff2r[:, 0:DTL, :])
                    wt = f1b[q]
                    for hc in range(DTL):
                        ht = q * DTL + hc
                        ps = pf1.tile([P, R], f32, tag="ph", bufs=2)
                        for dti in range(DTL):
                            nc.tensor.matmul(ps[:], wt[:, dti, bass.ts(hc, P)],
                                             x2T[:, dti, :],
                                             start=(dti == 0), stop=(dti == DTL - 1))
                        bias = 0.0 if trivial_ffb else ffb1h_t[:, ht:ht + 1]
                        nc.scalar.activation(hT[:, ht, :], ps[:], AF.Gelu,
                                             bias=bias, scale=1.0)

            with tc.tile_pool(name="ps_ff2", bufs=1, space="PSUM") as pf2:
                psl = [pf2.tile([P, 2, NK], f32, tag=f"pf{lt}", bufs=1,
                                name=f"pf{lt}") for lt in range(LT)]
                for q in range(4):
                    if q < 3:
                        # stream next ffW2 block; buffer freed by earlier reads
                        f2b[q + 1] = wblock(f"f2{q + 1}", ffW2_d,
                                            ff2r[:, (q + 1) * DTL:(q + 2) * DTL, :])
                    wt = f2b[q]
                    for hc in range(DTL):
                        ht = q * DTL + hc
                        for lt in range(LT):
                            for dc in range(2):
                                nc.tensor.matmul(
                                    psl[lt][:, dc, :],
                                    hT[:, ht, bass.ts(lt, P)],
                                    wt[:, hc, bass.ts(dc, NK)],
                                    start=(ht == 0), stop=(ht == HT - 1))
                for lt in range(LT):
                    rsb3 = small.tile([P, D], f32, tag="rsb", bufs=1)
                    for dc in range(2):
                        if trivial_ffb:
                            nc.vector.tensor_tensor(out=rsb3[:, bass.ts(dc, NK)],
                                                    in0=psl[lt][:, dc, :],
                                                    in1=x2[:, lt, bass.ts(dc, NK)],
                                                    op=ALU.add)
                        else:
                            ffb2c = small.tile([P, NK], f32, tag="ffb2", bufs=2)
                            nc.sync.dma_start(ffb2c[:], ffb2b_d[:, bass.ts(dc, NK)])
                            nc.vector.tensor_tensor(out=rsb3[:, bass.ts(dc, NK)],
                                                    in0=psl[lt][:, dc, :],
                                                    in1=ffb2c[:],
                                                    op=ALU.add)
                            nc.vector.tensor_tensor(out=rsb3[:, bass.ts(dc, NK)],
                                                    in0=rsb3[:, bass.ts(dc, NK)],
                                                    in1=x2[:, lt, bass.ts(dc, NK)],
                                                    op=ALU.add)
                    layernorm(rsb3, x3[:, lt, :], "3")
            nc.sync.dma_start(out_d.rearrange("(lt p) d -> p lt d", p=P), x3[:])

        sbP.release()
        small.release()
        cpool.release()

    lp.__exit__(None, None, None)
    nc.compile()
    return nc


def _host_prep(inputs):
    bfd = ml_dtypes.bfloat16
    x = np.ascontiguousarray(np.asarray(inputs["x"], np.float32))
    enc = np.ascontiguousarray(np.asarray(inputs["enc_output"], np.float32))
    mask = np.asarray(inputs["mask"])

    n = np.arange(D) // HD
    d = np.arange(D) % HD
    perm = d * H + n

    def pw(q, w, o):
        return (np.ascontiguousarray(np.asarray(q, np.float32)[:, perm].astype(bfd)),
                np.ascontiguousarray(np.asarray(w, np.float32)[:, perm].astype(bfd)),
                np.ascontiguousarray(np.asarray(o, np.float32)[perm, :].astype(bfd)))

    q1W, w1W, o1W = pw(inputs["q1W"], inputs["w1W"], inputs["o1W"])
    q2W, w2W, o2W = pw(inputs["q2W"], inputs["w2W"], inputs["o2W"])
    ffW1 = np.ascontiguousarray(np.asarray(inputs["ffW1"], np.float32).astype(bfd))
    ffW2 = np.ascontiguousarray(np.asarray(inputs["ffW2"], np.float32).astype(bfd))
    ffb1 = np.asarray(inputs["ffb1"], np.float32)
    ffb2 = np.asarray(inputs["ffb2"], np.float32)
    g = {k: np.asarray(inputs[k], np.float32)
         for k in ("g1", "b1", "g2", "b2", "g3", "b3")}

    trivial_affine = all(
        np.all(g[f"g{i}"] == 1.0) and np.all(g[f"b{i}"] == 0.0) for i in (1, 2, 3))
    trivial_ffb = bool(np.all(ffb1 == 0.0) and np.all(ffb2 == 0.0))

    mask01 = np.where(mask[:, 0, :, 0], np.float32(0.0),
                      np.float32(1.0)).astype(np.float32)

    xT = [np.ascontiguousarray(x[b].T.astype(bfd)) for b in range(B)]
    encT = [np.ascontiguousarray(enc[b].T.astype(bfd)) for b in range(B)]
    m01t = [np.ascontiguousarray(mask01[b].reshape(MT, P).T) for b in range(B)]
    m01b = [np.ascontiguousarray(
        np.broadcast_to(mask01[b].astype(bfd), (P, M))) for b in range(B)]

    in_maps = []
    for c in range(NCORES):
        b, grp = c // GROUPS, c % GROUPS
        im = {
            "xTq": np.ascontiguousarray(xT[b][:, grp * R:(grp + 1) * R]),
            "xT": xT[b],
            "encT": encT[b],
            "xrows": np.ascontiguousarray(x[b, grp * R:(grp + 1) * R, :]),
            "m01t": m01t[b], "m01b": m01b[b],
            "q1W": q1W, "w1W": w1W, "o1W": o1W,
            "q2W": q2W, "w2W": w2W, "o2W": o2W,
            "ffW1": ffW1, "ffW2": ffW2,
            "ffb1h": np.ascontiguousarray(ffb1.reshape(HT, P).T),
        }
        if not trivial_affine:
            for k in ("g1", "b1", "g2", "b2", "g3", "b3"):
                im[k + "b"] = np.ascontiguousarray(
                    np.broadcast_to(g[k], (P, D)).astype(np.float32))
        if not trivial_ffb:
            im["ffb2b"] = np.ascontiguousarray(
                np.broadcast_to(ffb2, (P, D)).astype(np.float32))
        in_maps.append(im)
    return in_maps, trivial_affine, trivial_ffb


def kernel(**inputs) -> np.ndarray:
    in_maps, trivial_affine, trivial_ffb = _host_prep(inputs)
    key = (trivial_affine, trivial_ffb)
    if key not in _PROGRAM_CACHE:
        _PROGRAM_CACHE[key] = _build_program(*key)
    nc = _PROGRAM_CACHE[key]
    res = run_bass_kernel_spmd(nc, in_maps, list(range(NCORES)))
    out = np.empty((B, L, D), np.float32)
    for c in range(NCORES):
        b, grp = c // GROUPS, c % GROUPS
        out[b, grp * R:(grp + 1) * R, :] = res.results[c]["out"]
    return out
